# revision 1
# baseline (speedup 1.0000x reference)
"""Trainium2 Bass kernel for nn_DeltaEncoderBlock.

Reference semantics (all fp32):
    x: [64, 9, 14, 384] -> x_flat [64, 126, 384]
    delta[t] = x[t] - x[t-1]  (delta[0] = x[0])        (temporal delta)
    w = g * v / ||v||_row                               (weight norm, [1024, 126])
    z = einsum('oi,bit->tbo', w, delta)                 (synaptic input)
    scan over t:  cur = 0.75*cur + z_t
                  vol = 0.97*vol + cur
                  s   = (vol >= 1)
                  vol = vol * (1 - s)                   (hard reset)
    out: spikes [64, 1024, 384]

Sharding: data-parallel over batch across 8 NeuronCores (8 batches/core).

Per-core kernel:
  - z via PE fp32 matmuls (K=126), o in 8 chunks of 128 partitions,
    weight-norm scale applied in the PSUM->SBUF copy on ScalarE.
  - cur via DVE tensor_tensor_scan (linear recurrence along t).
  - vol/spike loop: 2 fused scalar_tensor_tensor DVE ops per step;
    spike = Relu(Sign(vol_pre - 1)) on ScalarE (Sign per step, Relu per
    48-step block), DMA'd out per block.
"""

import numpy as np

import concourse.bacc as bacc
import concourse.tile as tile
from concourse import mybir
from concourse.bass_utils import run_bass_kernel_spmd
from concourse.masks import make_identity

N_CORES = 8
B, C, H, T = 64, 9, 14, 384
I = C * H  # 126
O = 1024
BL = B // N_CORES  # 8 batches per core
NCH = O // 128  # 8 o-chunks of 128
TBLK = 64  # t-block: matmul window, z tile span, and spike staging block
NTB = T // TBLK  # 6
F32 = mybir.dt.float32
U8 = mybir.dt.uint8

CURRENT_DECAY = 0.25
VOLTAGE_DECAY = 0.03

# fp32r streams fp32 through the PE at bf16 rate (4x faster than plain fp32
# matmul); numerics differ slightly from fp32 — gated on a HW accuracy check.
MM_F32R = False


def _body(tc, x, v, g, out):
    nc = tc.nc
    Alu = mybir.AluOpType
    Act = mybir.ActivationFunctionType

    import contextlib

    with contextlib.ExitStack() as ctx:
        consts = ctx.enter_context(tc.tile_pool(name="consts", bufs=1))
        big = ctx.enter_context(tc.tile_pool(name="big", bufs=1))
        wp = ctx.enter_context(tc.tile_pool(name="wp", bufs=1))
        psT = ctx.enter_context(tc.tile_pool(name="psT", bufs=2, space="PSUM"))
        psZ = ctx.enter_context(tc.tile_pool(name="psZ", bufs=6, space="PSUM"))
        pvolS = ctx.enter_context(tc.tile_pool(name="pvolS", bufs=8))
        pstage = ctx.enter_context(tc.tile_pool(name="pstage", bufs=2))

        # ---- front-load the weight path: the norm chain (v DMA -> Squares
        # -> rsqrt -> scale) gates the first z copies and thus the loop
        # start, so its DMAs go on the queue FIRST and both ACT function
        # tables (Square/... and Sqrt) are loaded by dummy ops at t~0. ----
        decD = consts.tile([I, 1], F32)
        nc.vector.memset(decD[:], 1.0 - CURRENT_DECAY)
        neg1 = consts.tile([128, 1], F32)
        nc.vector.memset(neg1[:], -1.0)
        actwarm = consts.tile([128, 1], F32)
        nc.scalar.activation(actwarm[:], neg1[:], Act.Square)
        nc.scalar.activation(actwarm[:], actwarm[:], Act.Sqrt)

        vt = wp.tile([128, NCH * I], F32)
        vt3 = vt[:].rearrange("p (c i) -> p c i", c=NCH)
        nc.sync.dma_start(vt3, v.rearrange("(c p) i -> p c i", p=128))
        gt = wp.tile([128, NCH], F32)
        nc.sync.dma_start(gt[:], g.rearrange("(c p) -> p c", p=128))

        sq = wp.tile([128, I], F32)  # scratch for Square
        ss = wp.tile([128, NCH], F32)  # row sum-of-squares
        for c in range(NCH):
            nc.scalar.activation(
                sq[:], vt3[:, c, :], Act.Square, accum_out=ss[:, c : c + 1]
            )
        inv = wp.tile([128, NCH], F32)
        nc.vector.reciprocal(inv[:], ss[:])
        rs = wp.tile([128, NCH], F32)
        nc.scalar.sqrt(rs[:], inv[:])  # rsqrt(sum v^2)
        scale = wp.tile([128, NCH], F32)
        nc.vector.tensor_tensor(out=scale[:], in0=rs[:], in1=gt[:], op=Alu.mult)

        # ---- load x per batch, pipelined with delta + cur-delta scan ----
        # cur-delta: scan the 0.75 recurrence on delta before the matmul;
        # W.(scan delta) == scan (W.delta) by linearity.
        xs = big.tile([I, BL * T], F32)
        x3 = xs[:].rearrange("p (b t) -> p b t", b=BL)
        delta = big.tile([I, BL * T], F32)
        d3 = delta[:].rearrange("p (b t) -> p b t", b=BL)
        TH = 2 * TBLK  # scan in halves: first half unblocks early matmuls
        for b in range(BL):
            nc.sync.dma_start(x3[:, b, :], x[b].rearrange("i t -> i t"))
            nc.vector.tensor_copy(d3[:, b, 0:1], x3[:, b, 0:1])
            nc.vector.tensor_tensor(
                out=d3[:, b, 1:TH], in0=x3[:, b, 1:TH], in1=x3[:, b, 0 : TH - 1],
                op=Alu.subtract,
            )
            seg = delta[:, b * T : b * T + TH]
            nc.vector.tensor_tensor_scan(
                seg, decD[:].to_broadcast([I, TH]), seg, 0.0, Alu.mult, Alu.add
            )
        for b in range(BL):
            nc.vector.tensor_tensor(
                out=d3[:, b, TH:T], in0=x3[:, b, TH:T], in1=x3[:, b, TH - 1 : T - 1],
                op=Alu.subtract,
            )
            seg = delta[:, b * T + TH : (b + 1) * T]
            carry = delta[:, b * T + TH - 1 : b * T + TH]
            nc.vector.tensor_tensor_scan(
                seg, decD[:].to_broadcast([I, T - TH]), seg, carry,
                Alu.mult, Alu.add,
            )

        ident = consts.tile([128, 128], F32)
        make_identity(nc, ident[:])

        # PE HAM warm-up: dummy matmuls during the input DMA so the real
        # matmuls run at 2.4GHz from the start (HAM un-throttles after
        # ~3.4us of sustained PE activity). Results are never read; real
        # matmuls use start=True so the shared PSUM slots are reset.
        for _ in range(16):
            wps = psZ.tile([128, BL * TBLK], F32, tag="ps")
            nc.tensor.matmul(
                wps[:, 0:128], lhsT=ident[:], rhs=ident[:],
                start=True, stop=True,
            )

        wT = []  # per-chunk [126, 128] tiles of v^T
        for c in range(NCH):
            pt = psT.tile([I, 128], F32)
            nc.tensor.transpose(pt[:], vt3[:, c, :], ident[:])
            wc = wp.tile([I, 128], F32, tag=f"wT{c}")
            nc.scalar.copy(wc[:], pt[:])
            wT.append(wc)

        # ---- cur = (v^T . cur-delta), scaled by g/||v|| on the PSUM->SBUF
        # copy. One z tile per t-block of TBLK steps, layout [p, (c b tl)],
        # so the vol loop starts after the first t-block's matmuls and the
        # rest of the matmul phase hides under the loop. Matmul windows
        # enumerate (tl, b) columns via a strided rhs AP on delta. ----
        dly = delta[:].rearrange("p (b t) -> p t b", b=BL)  # [126, T, BL]
        ztiles = []
        for tb in range(NTB):
            zt = big.tile([128, NCH * BL * TBLK], F32, tag=f"z{tb}")
            ztiles.append(zt)
            # memory layout (c, b, tl); dims permuted to enumerate (tl, b)
            zv = zt[:].rearrange("p (c b tl) -> p c tl b", c=NCH, b=BL)
            # first t-block in half-windows: the loop's first steps gate on
            # 8 half-size matmuls instead of 8 full ones (subtile deps let
            # A(t<32) start once the first halves are copied).
            halves = ((0, TBLK // 2), (TBLK // 2, TBLK)) if tb == 0 else (
                (0, TBLK),
            )
            for wlo, whi in halves:
                ww = whi - wlo
                for c in range(NCH):
                    ps = psZ.tile([128, BL * TBLK], F32, tag="ps")
                    mm_lhs = wT[c][:]
                    mm_rhs = dly[:, tb * TBLK + wlo : tb * TBLK + whi, :]
                    if MM_F32R:
                        mm_lhs = mm_lhs.bitcast(mybir.dt.float32r)
                        mm_rhs = mm_rhs.bitcast(mybir.dt.float32r)
                    nc.tensor.matmul(
                        ps[:, : ww * BL], lhsT=mm_lhs, rhs=mm_rhs,
                        start=True, stop=True,
                    )
                    # psum cols are (tl, b); write to z at (b*TBLK + tl)
                    nc.scalar.activation(
                        zv[:, c, wlo:whi, :],
                        ps[:, : ww * BL].rearrange(
                            "p (tl b) -> p tl b", b=BL
                        ),
                        Act.Copy,
                        scale=scale[:, c : c + 1],
                    )

        # ---- vol loop: vol_pre overwrites the cur column of z in place.
        # DVE-only; no cross-engine sync inside the loop. ----
        volS = None
        vdec = 1.0 - VOLTAGE_DECAY

        out_r = out.rearrange("b (c p) t -> p c b t", c=NCH)
        for t in range(T):
            tb, tl = divmod(t, TBLK)
            zc = ztiles[tb][:].rearrange(
                "p (c b tl) -> p c b tl", c=NCH, b=BL
            )
            # vol_pre = vdec * vol + cur_t   (written over cur_t).
            # t=0: vol_pre = cur_0 is already in place — skip the op.
            if t > 0:
                nc.vector.scalar_tensor_tensor(
                    zc[:, :, :, tl],
                    volS[:].rearrange("p (c b) -> p c b", c=NCH),
                    vdec,
                    zc[:, :, :, tl],
                    Alu.mult,
                    Alu.add,
                )
            # vol = (vol_pre < 1) * vol_pre   (hard reset); the state after
            # the last step is never consumed — skip it.
            if t < T - 1:
                volS = pvolS.tile([128, NCH * BL], F32, tag="volS")
                nc.vector.scalar_tensor_tensor(
                    volS[:].rearrange("p (c b) -> p c b", c=NCH),
                    zc[:, :, :, tl],
                    1.0,
                    zc[:, :, :, tl],
                    Alu.is_lt,
                    Alu.mult,
                )
            # spikes on ACT (off the DVE path): Sign -> in-place Relu ->
            # DMA out, staging (c, b, tl). Block tb-1 is extracted at the
            # START of block tb (its data is complete and the ACT ops are
            # immediately ready); the final block flushes in quarters as
            # its columns finish so only the last quarter sits on the tail.
            flush = []
            if tb >= 1 and tl == 0:
                flush = [(tb - 1, 0, TBLK)]
            if tb == NTB - 1 and (tl + 1) % (TBLK // 4) == 0:
                q = (tl + 1) // (TBLK // 4) - 1
                flush += [(tb, q * (TBLK // 4), (q + 1) * (TBLK // 4))]
            for ftb, lo, hi in flush:
                w = hi - lo
                zcf = ztiles[ftb][:].rearrange(
                    "p (c b tl) -> p c b tl", c=NCH, b=BL
                )
                ostage = pstage.tile([128, 64 * w], U8, tag=f"os{w}")
                o3 = ostage[:].rearrange(
                    "p (c b tl) -> p c b tl", c=NCH, b=BL
                )
                # spikes are exact 0/1: Sign then Relu (which also narrows
                # to uint8 -> 4x fewer DMA bytes; host widens). A DVE
                # is_ge->uint8 shortcut for the final quarter matched in
                # CoreSim but was WRONG on hardware — keep ACT.
                sstage = pstage.tile([128, 64 * w], F32, tag=f"ss{w}")
                s3 = sstage[:].rearrange(
                    "p (c b tl) -> p c b tl", c=NCH, b=BL
                )
                nc.scalar.activation(
                    s3, zcf[:, :, :, lo:hi], Act.Sign, bias=neg1[:]
                )
                nc.scalar.activation(o3, s3, Act.Relu)
                for c in range(NCH):
                    nc.sync.dma_start(
                        out_r[:, c, :, ftb * TBLK + lo : ftb * TBLK + hi],
                        o3[:, c, :, :],
                    )


_CACHE = {}


def _build():
    if "nc" in _CACHE:
        return _CACHE["nc"]
    nc = bacc.Bacc(
        "TRN2", target_bir_lowering=False, debug=False, num_devices=N_CORES
    )
    x = nc.dram_tensor("x", [BL, I, T], F32, kind="ExternalInput").ap()
    v = nc.dram_tensor("v", [O, I], F32, kind="ExternalInput").ap()
    g = nc.dram_tensor("g", [O], F32, kind="ExternalInput").ap()
    out = nc.dram_tensor("out", [BL, O, T], U8, kind="ExternalOutput").ap()
    with tile.TileContext(nc) as tc:
        _body(tc, x, v, g, out)
    nc.compile()
    _CACHE["nc"] = nc
    return nc


def make_in_maps(x, v_weight, g):
    xr = np.ascontiguousarray(x.reshape(B, I, T))
    v_weight = np.ascontiguousarray(v_weight)
    g = np.ascontiguousarray(g)
    return [
        {
            "x": np.ascontiguousarray(xr[c * BL : (c + 1) * BL]),
            "v": v_weight,
            "g": g,
        }
        for c in range(N_CORES)
    ]


def kernel(x, v_weight, g):
    nc = _build()
    in_maps = make_in_maps(
        np.asarray(x, dtype=np.float32),
        np.asarray(v_weight, dtype=np.float32),
        np.asarray(g, dtype=np.float32),
    )
    last_err = None
    for _attempt in range(3):  # retry: a prior tenant can leave a core wedged
        try:
            res = run_bass_kernel_spmd(nc, in_maps, list(range(N_CORES))).results
            return np.concatenate(
                [res[c]["out"] for c in range(N_CORES)], axis=0
            ).astype(np.float32)
        except Exception as e:  # noqa: BLE001
            last_err = e
    raise last_err



# revision 69
# speedup vs baseline: 1.2010x; 1.2010x over previous
"""Trainium2 Bass kernel for nn_DeltaEncoderBlock.

Reference semantics (all fp32):
    x: [64, 9, 14, 384] -> x_flat [64, 126, 384]
    delta[t] = x[t] - x[t-1]  (delta[0] = x[0])        (temporal delta)
    w = g * v / ||v||_row                               (weight norm, [1024, 126])
    z = einsum('oi,bit->tbo', w, delta)                 (synaptic input)
    scan over t:  cur = 0.75*cur + z_t
                  vol = 0.97*vol + cur
                  s   = (vol >= 1)
                  vol = vol * (1 - s)                   (hard reset)
    out: spikes [64, 1024, 384]

Sharding: data-parallel over batch across 8 NeuronCores (8 batches/core).

Per-core kernel:
  - z via PE fp32 matmuls (K=126), o in 8 chunks of 128 partitions,
    weight-norm scale applied in the PSUM->SBUF copy on ScalarE.
  - cur via DVE tensor_tensor_scan (linear recurrence along t).
  - vol/spike loop: 2 fused scalar_tensor_tensor ops per step; the 64
    state columns are split into two independent serial chains that run
    concurrently: DVE takes o-chunks [0, NDV), GpSimd/Pool takes
    [NDV, 8).  Each chain is latency-bound (~110ns fixed + width
    cycles per op), so narrowing the per-op width via the split cuts
    the per-step critical path.
  - spike = Relu(Sign(vol_pre - 1)) on ScalarE (Sign per step, Relu per
    48-step block), DMA'd out per block.
"""

import numpy as np

import concourse.bacc as bacc
import concourse.tile as tile
from concourse import mybir
from concourse.bass_utils import run_bass_kernel_spmd
from concourse.masks import make_identity

N_CORES = 8
B, C, H, T = 64, 9, 14, 384
I = C * H  # 126
O = 1024
BL = B // N_CORES  # 8 batches per core
NCH = O // 128  # 8 o-chunks of 128
TBLK = 64  # t-block: matmul window, z tile span, and spike staging block
NTB = T // TBLK  # 6
TSEG = 16  # spike output segment (one DMA per TSEG steps)
# Output segment schedule: uniform 16-step segments, except the final 16
# steps go out as two 8-step segments so only ~8 steps of extraction+DMA
# sit on the kernel tail.  Device out is a flat [128, 64*T] u8 buffer,
# written contiguously segment by segment in (b, c, tl) order.
SEGS = [(s, s + TSEG) for s in range(0, T - TSEG, TSEG)] + [
    (T - TSEG, T - 4),
    (T - 4, T),
]
SEG_END = {hi: lo for lo, hi in SEGS}
F32 = mybir.dt.float32
U8 = mybir.dt.uint8

CURRENT_DECAY = 0.25
VOLTAGE_DECAY = 0.03

# Column split of the vol loop's 64 state columns (flattened (chunk,
# batch)) into two independent serial chains on DVE: A [0, CA) and
# B [CA, 64), interleaved 1A 1B 2A 2B so B's exec hides A's SBUF
# write-ack latency.  (GpSimd cannot run scalar_tensor_tensor — the
# neuronxcc backend rejects TensorScalarPtr on Pool — so the serial
# chains are DVE-only; Pool still runs the delta subtracts.)
CA = 32

# fp32r streams fp32 through the PE at bf16 rate (4x faster than plain fp32
# matmul); numerics differ slightly from fp32 — gated on a HW accuracy check.
MM_F32R = False


def _body(tc, x, v, g, out):
    nc = tc.nc
    Alu = mybir.AluOpType
    Act = mybir.ActivationFunctionType

    import contextlib

    with contextlib.ExitStack() as ctx:
        consts = ctx.enter_context(tc.tile_pool(name="consts", bufs=1))
        big = ctx.enter_context(tc.tile_pool(name="big", bufs=1))
        wp = ctx.enter_context(tc.tile_pool(name="wp", bufs=1))
        psT = ctx.enter_context(tc.tile_pool(name="psT", bufs=2, space="PSUM"))
        psZ = ctx.enter_context(tc.tile_pool(name="psZ", bufs=6, space="PSUM"))
        pvolS = ctx.enter_context(tc.tile_pool(name="pvolS", bufs=8))
        pstage = ctx.enter_context(tc.tile_pool(name="pstage", bufs=4))

        # ---- constants + single ACT table: Square/Rsqrt/Copy/Sign/Relu all
        # live in the 'reciprocal_sqrt_and_small' ACT function set, so the
        # two warm-up ops below trigger exactly ONE LoadActFuncSet. ----
        decD = consts.tile([I, 1], F32)
        nc.vector.memset(decD[:], 1.0 - CURRENT_DECAY)
        neg1 = consts.tile([128, 1], F32)
        nc.vector.memset(neg1[:], -1.0)
        actwarm = consts.tile([128, 1], F32)
        nc.scalar.activation(actwarm[:], neg1[:], Act.Square)
        nc.scalar.activation(actwarm[:], actwarm[:], Act.Sqrt)

        # ---- DMA queue order (transfers serialize on the one HW queue, so
        # order = priority): x[t<64] unblocks the first-half delta prep, the
        # two v halves unblock transposes + squares, then the rest of x in
        # two chunks so the GpSimd second-half prep can start early. ----
        xs = big.tile([I, BL * T], F32)
        x3 = xs[:].rearrange("p (b t) -> p b t", b=BL)
        delta = big.tile([I, BL * T], F32)
        d3 = delta[:].rearrange("p (b t) -> p b t", b=BL)
        TH = TBLK // 2  # DVE preps t < TH; GpSimd subs the rest
        T2 = 224  # split point of the GpSimd subs / x tail DMA
        # all inputs on the SP DMA queue, in criticality order (the model
        # serializes all DMA transfers): x[t<64] first (unblocks delta
        # prep), then the v halves (squares/transposes), then the x tail
        xr = x.rearrange("b i t -> i b t")
        vt = wp.tile([128, NCH * I], F32)
        vt3 = vt[:].rearrange("p (c i) -> p c i", c=NCH)
        vr = v.rearrange("(c p) i -> p c i", p=128)
        gt = wp.tile([128, NCH], F32)
        nc.sync.dma_start(x3[:, :, 0:TH], xr[:, :, 0:TH])
        nc.sync.dma_start(vt3[:, 0 : NCH // 2, :], vr[:, 0 : NCH // 2, :])
        nc.sync.dma_start(vt3[:, NCH // 2 :, :], vr[:, NCH // 2 :, :])
        nc.sync.dma_start(gt[:], g.rearrange("(c p) -> p c", p=128))
        nc.sync.dma_start(x3[:, :, TH:T2], xr[:, :, TH:T2])
        nc.sync.dma_start(x3[:, :, T2:T], xr[:, :, T2:T])

        # ---- first-block delta + cur-delta scan (cur-delta: scan the
        # 0.75 recurrence on delta before the matmul; W.(scan delta) ==
        # scan (W.delta) by linearity).  Both on DVE, interleaved per
        # batch: the scan follows its subtract in-order with no
        # cross-engine semaphore, so the first-block prep finishes ~1.2us
        # sooner than with the subtracts on GpSimd. ----
        for b in range(BL):
            nc.vector.tensor_copy(d3[:, b, 0:1], x3[:, b, 0:1])
            nc.vector.tensor_tensor(
                out=d3[:, b, 1:TH], in0=x3[:, b, 1:TH], in1=x3[:, b, 0 : TH - 1],
                op=Alu.subtract,
            )
            seg = delta[:, b * T : b * T + TH]
            nc.vector.tensor_tensor_scan(
                seg, decD[:].to_broadcast([I, TH]), seg, 0.0, Alu.mult, Alu.add
            )
        # ---- weight-norm chain: squares on DVE (right after the delta
        # first-half, in parallel with the ACT wT copies), rsqrt in one ACT
        # op, final scale mult back on DVE. ----
        ss = wp.tile([128, NCH], F32)  # row sum-of-squares
        sq = wp.tile([128, I], F32)  # scratch
        for c in range(NCH):
            nc.vector.scalar_tensor_tensor(
                sq[:], vt3[:, c, :], 1.0, vt3[:, c, :], Alu.mult, Alu.mult,
                accum_out=ss[:, c : c + 1],
            )
        ident = consts.tile([128, 128], F32)
        make_identity(nc, ident[:])  # pool op, emitted before the pool prep

        # PE HAM warm-up: dummy matmuls during the input DMA so the real
        # matmuls run at 2.4GHz from the start (HAM un-throttles after
        # ~3.4us of sustained PE activity; 8 x 427ns covers it). PE is
        # in-order, so more warm-ups would delay the wT transposes.
        for _ in range(8):
            wps = psZ.tile([128, BL * TBLK], F32, tag="ps")
            nc.tensor.matmul(
                wps[:, 0:128], lhsT=ident[:], rhs=ident[:],
                start=True, stop=True,
            )

        # wT transposes + copies, with the norm-chain tail WEDGED between
        # the first and second half of the wc copies: reciprocal on DVE
        # (Act.Rsqrt is rejected by bass for HW accuracy), sqrt on ACT
        # right after wc c0-3 (ACT is in-order; emitting sqrt after all 8
        # wc copies would delay `scale` and the first z copies), scale
        # mult on DVE.
        inv = wp.tile([128, NCH], F32)
        rs = wp.tile([128, NCH], F32)
        scale = wp.tile([128, NCH], F32)
        wT = []  # per-chunk [126, 128] tiles of v^T
        for c in range(NCH):
            pt = psT.tile([I, 128], F32)
            nc.tensor.transpose(pt[:], vt3[:, c, :], ident[:])
            wc = wp.tile([I, 128], F32, tag=f"wT{c}")
            nc.scalar.copy(wc[:], pt[:])
            wT.append(wc)
        nc.vector.reciprocal(inv[:], ss[:])
        nc.scalar.sqrt(rs[:], inv[:])  # rsqrt(sum v^2)
        nc.vector.tensor_tensor(
            out=scale[:], in0=rs[:], in1=gt[:], op=Alu.mult
        )

        # ---- rest of the delta SUBTRACTS on GpSimd, in two t-ranges keyed
        # to the two x tail DMAs.  The matching scans are DVE-only; they
        # are emitted lazily inside the vol loop (see _pending_scans) so
        # they don't delay the loop start — block-1 matmuls don't need
        # them until ~25us into the loop.
        for rlo, rhi in ((TH, T2), (T2, T)):
            for b in range(BL):
                nc.gpsimd.tensor_tensor(
                    out=d3[:, b, rlo:rhi],
                    in0=x3[:, b, rlo:rhi],
                    in1=x3[:, b, rlo - 1 : rhi - 1],
                    op=Alu.subtract,
                )

        def _emit_tail_scan(b, rlo, rhi):
            seg = delta[:, b * T + rlo : b * T + rhi]
            carry = delta[:, b * T + rlo - 1 : b * T + rlo]
            nc.vector.tensor_tensor_scan(
                seg, decD[:].to_broadcast([I, rhi - rlo]), seg, carry,
                Alu.mult, Alu.add,
            )

        # ---- cur = (v^T . cur-delta), scaled by g/||v|| on the PSUM->SBUF
        # copy. One z tile per t-block of TBLK steps, layout [p, (c b tl)].
        # Matmul windows enumerate (tl, b) columns via a strided rhs AP on
        # delta.  Block 0 is emitted before the vol loop (in quarters, the
        # first quarter's copies on DVE); blocks 1+ are emitted a few loop
        # steps in, AFTER the lazily-emitted tail scans that produce their
        # rhs data. ----
        dly = delta[:].rearrange("p (b t) -> p t b", b=BL)  # [126, T, BL]
        ztiles = [
            big.tile([128, NCH * BL * TBLK], F32, tag=f"z{tb}", name=f"zt{tb}")
            for tb in range(NTB)
        ]

        def emit_zblock(tb, windows):
            zv = ztiles[tb][:].rearrange(
                "p (c b tl) -> p c tl b", c=NCH, b=BL
            )
            for wlo, whi in windows:
                ww = whi - wlo
                for c in range(NCH):
                    ps = psZ.tile([128, BL * TBLK], F32, tag="ps")
                    mm_lhs = wT[c][:]
                    mm_rhs = dly[:, tb * TBLK + wlo : tb * TBLK + whi, :]
                    if MM_F32R:
                        mm_lhs = mm_lhs.bitcast(mybir.dt.float32r)
                        mm_rhs = mm_rhs.bitcast(mybir.dt.float32r)
                    nc.tensor.matmul(
                        ps[:, : ww * BL], lhsT=mm_lhs, rhs=mm_rhs,
                        start=True, stop=True,
                    )
                    # psum cols are (tl, b); write to z at (b*TBLK + tl).
                    # The very first quarter's copies run on DVE (in its
                    # queue right before the vol loop, in parallel with the
                    # ACT wc copies) so the loop isn't gated on the ACT
                    # queue draining; everything else copies on ACT.
                    ps_v = ps[:, : ww * BL].rearrange("p (tl b) -> p tl b", b=BL)
                    if tb == 0 and wlo == 0 and c < NCH // 2:
                        nc.vector.tensor_scalar_mul(
                            zv[:, c, wlo:whi, :], ps_v, scale[:, c : c + 1]
                        )
                    else:
                        nc.scalar.activation(
                            zv[:, c, wlo:whi, :], ps_v,
                            Act.Copy, scale=scale[:, c : c + 1],
                        )

        # only the first half of block 0 (t < TH = 32) has scanned delta
        # before the loop starts; the second half's windows are emitted a
        # few steps into the loop, after the first injected scan.
        emit_zblock(0, ((0, 16), (16, 32)))

        # ---- vol loop: vol_pre overwrites the cur column of z in place.
        # Two independent serial chains over the 64 flattened (chunk,
        # batch) state columns, both on DVE (A [0, CA), B [CA, 64)),
        # interleaved 1A 1B 2A 2B: B's exec hides A's write-ack latency.
        groups = [
            ("A", 0, CA, nc.vector),
            ("B", CA, 64, nc.vector),
        ]
        volS = {g: None for g, _, _, _ in groups}
        vdec = 1.0 - VOLTAGE_DECAY

        # out DRAM layout is [p, ts, b, c, tl16] (host re-transposes): a
        # 16-step flush then lands as ONE DMA whose (b c tl) span is
        # contiguous per partition -> 128 descriptors instead of 8192.
        out5 = out

        def emit_step(t):
            tb, tl = divmod(t, TBLK)
            zcb = ztiles[tb][:].rearrange("p (cb tl) -> p cb tl", cb=64)
            # vol_pre = vdec * vol + cur_t   (written over cur_t).
            # t=0: vol_pre = cur_0 is already in place — skip the op.
            if t > 0:
                for g, lo, hi, eng in groups:
                    eng.scalar_tensor_tensor(
                        zcb[:, lo:hi, tl],
                        volS[g][:],
                        vdec,
                        zcb[:, lo:hi, tl],
                        Alu.mult,
                        Alu.add,
                    )
            # vol = (vol_pre < 1) * vol_pre   (hard reset); the state after
            # the last step is never consumed — skip it.
            if t < T - 1:
                for g, lo, hi, eng in groups:
                    vt = pvolS.tile([128, hi - lo], F32, tag=f"volS{g}")
                    volS[g] = vt
                    eng.scalar_tensor_tensor(
                        vt[:],
                        zcb[:, lo:hi, tl],
                        1.0,
                        zcb[:, lo:hi, tl],
                        Alu.is_lt,
                        Alu.mult,
                    )
            # spikes on ACT (off the DVE path): at each segment boundary,
            # Sign -> Relu -> one 128-descriptor DMA of the finished
            # segment (contiguous (b c tl) span in the flat out buffer).
            if (t + 1) in SEG_END:
                slo = SEG_END[t + 1]
                w = t + 1 - slo
                lo = slo - tb * TBLK
                hi = lo + w
                zcf = ztiles[tb][:].rearrange(
                    "p (c b tl) -> p c b tl", c=NCH, b=BL
                )
                ostage = pstage.tile([128, 64 * w], U8, tag=f"os{w}")
                o3 = ostage[:].rearrange(
                    "p (b c tl) -> p b c tl", b=BL, c=NCH
                )
                # spikes are exact 0/1: Sign then Relu (which also narrows
                # to uint8 -> 4x fewer DMA bytes; host widens). A DVE
                # is_ge->uint8 shortcut matched in CoreSim but was WRONG
                # on hardware — keep ACT.
                sstage = pstage.tile([128, 64 * w], F32, tag=f"ss{w}")
                s3 = sstage[:].rearrange(
                    "p (b c tl) -> p c b tl", b=BL, c=NCH
                )
                nc.scalar.activation(
                    s3, zcf[:, :, :, lo:hi], Act.Sign, bias=neg1[:]
                )
                nc.scalar.activation(ostage[:], sstage[:], Act.Relu)
                nc.sync.dma_start(out5[:, 64 * slo : 64 * (t + 1)], o3)

        # The rest of the work is interleaved into the loop emission, in
        # dependency order: the 16 tail scans are injected one per two
        # steps ([TH,T2) as soon as its x DMA + subs can land, [T2,T)
        # later); the second half of z block 0, then block 1, then blocks
        # 2-5 are emitted once the scans covering their rhs are in.
        SPLIT_0B = 11
        SPLIT_B1 = 13
        SPLIT_REST = 22
        for t in range(T):
            emit_step(t)
            if 2 <= t < 2 + BL:
                _emit_tail_scan(t - 2, TH, T2)
            if 12 <= t < 12 + BL:
                _emit_tail_scan(t - 12, T2, T)
            if t == SPLIT_0B:
                emit_zblock(0, ((32, 48), (48, 64)))
            if t == SPLIT_B1:
                emit_zblock(1, ((0, TBLK),))
            if t == SPLIT_REST:
                for tb in range(2, NTB):
                    emit_zblock(tb, ((0, TBLK),))


_CACHE = {}


def _build():
    if "nc" in _CACHE:
        return _CACHE["nc"]
    nc = bacc.Bacc(
        "TRN2", target_bir_lowering=False, debug=False, num_devices=N_CORES
    )
    x = nc.dram_tensor("x", [BL, I, T], F32, kind="ExternalInput").ap()
    v = nc.dram_tensor("v", [O, I], F32, kind="ExternalInput").ap()
    g = nc.dram_tensor("g", [O], F32, kind="ExternalInput").ap()
    out = nc.dram_tensor(
        "out", [128, 64 * T], U8, kind="ExternalOutput"
    ).ap()
    with tile.TileContext(nc) as tc:
        _body(tc, x, v, g, out)
    nc.compile()
    _CACHE["nc"] = nc
    return nc


def make_in_maps(x, v_weight, g):
    xr = np.ascontiguousarray(x.reshape(B, I, T))
    v_weight = np.ascontiguousarray(v_weight)
    g = np.ascontiguousarray(g)
    return [
        {
            "x": np.ascontiguousarray(xr[c * BL : (c + 1) * BL]),
            "v": v_weight,
            "g": g,
        }
        for c in range(N_CORES)
    ]


def kernel(x, v_weight, g):
    nc = _build()
    in_maps = make_in_maps(
        np.asarray(x, dtype=np.float32),
        np.asarray(v_weight, dtype=np.float32),
        np.asarray(g, dtype=np.float32),
    )
    last_err = None
    for _attempt in range(3):  # retry: a prior tenant can leave a core wedged
        try:
            res = run_bass_kernel_spmd(nc, in_maps, list(range(N_CORES))).results
            # device out is a flat [128, 64*T] buffer of (b, c, tl)
            # segments per SEGS; host untransposes to [b, o=c*128+p, t]
            parts = []
            for core in range(N_CORES):
                arr = res[core]["out"]  # [128, 64*T] u8
                full = np.empty((BL, O, T), np.uint8)
                o_view = full.reshape(BL, NCH, 128, T)
                for lo, hi in SEGS:
                    w = hi - lo
                    seg = arr[:, 64 * lo : 64 * hi].reshape(128, BL, NCH, w)
                    # seg[p, b, c, tl] -> o_view[b, c, p, lo:hi]
                    o_view[:, :, :, lo:hi] = np.transpose(seg, (1, 2, 0, 3))
                parts.append(full)
            return np.concatenate(parts, axis=0).astype(np.float32)
        except Exception as e:  # noqa: BLE001
            last_err = e
    raise last_err



# revision 73
# speedup vs baseline: 1.2012x; 1.0001x over previous
"""Trainium2 Bass kernel for nn_DeltaEncoderBlock.

Reference semantics (all fp32):
    x: [64, 9, 14, 384] -> x_flat [64, 126, 384]
    delta[t] = x[t] - x[t-1]  (delta[0] = x[0])        (temporal delta)
    w = g * v / ||v||_row                               (weight norm, [1024, 126])
    z = einsum('oi,bit->tbo', w, delta)                 (synaptic input)
    scan over t:  cur = 0.75*cur + z_t
                  vol = 0.97*vol + cur
                  s   = (vol >= 1)
                  vol = vol * (1 - s)                   (hard reset)
    out: spikes [64, 1024, 384]

Sharding: data-parallel over batch across 8 NeuronCores (8 batches/core).

Per-core kernel:
  - z via PE fp32 matmuls (K=126), o in 8 chunks of 128 partitions,
    weight-norm scale applied in the PSUM->SBUF copy on ScalarE.
  - cur via DVE tensor_tensor_scan (linear recurrence along t), folded
    into the input (delta) side by linearity: W.(scan delta) ==
    scan (W.delta).
  - vol/spike loop (the critical path, ~145us of ~165): per step, two
    scalar_tensor_tensor ops (vol_pre = 0.97*vol + cur, then the
    gate/reset vol' = (vol_pre < 1)*vol_pre).  The 64 state columns are
    split into TWO independent serial chains on DVE, interleaved
    1A 1B 2A 2B: B's exec hides A's ~110ns SBUF write-ack + issue
    latency, making the loop ENGINE-bound at ~375ns/step (4 ops x
    ~94ns) instead of latency-bound at ~475ns/step.  GpSimd cannot
    take a chain: neuronxcc rejects TensorScalarPtr (STT and
    tensor_tensor_scan) on Pool; Pool only runs TensorTensor /
    TensorScalar / TensorCopy, which would need 3+ ops per step.
  - spike = Relu(Sign(vol_pre - 1)) on ScalarE every 16 steps, staged
    (b, c, t) and DMA'd as ONE contiguous 128-descriptor chunk of a
    flat [128, 64*T] u8 out buffer (the host un-transposes); the final
    16 steps flush as 12+4 so little extraction sits on the tail.
  - startup: DMA priority order x[t<32] -> v -> x-tail; first-block
    delta prep + weight-norm squares/rsqrt chain on DVE; block-0
    matmul windows in quarters; the remaining delta scans and z blocks
    are emitted a few steps INTO the loop, in dependency order.
"""

import numpy as np

import concourse.bacc as bacc
import concourse.tile as tile
from concourse import mybir
from concourse.bass_utils import run_bass_kernel_spmd
from concourse.masks import make_identity

N_CORES = 8
B, C, H, T = 64, 9, 14, 384
I = C * H  # 126
O = 1024
BL = B // N_CORES  # 8 batches per core
NCH = O // 128  # 8 o-chunks of 128
TBLK = 64  # t-block: matmul window, z tile span, and spike staging block
NTB = T // TBLK  # 6
TSEG = 16  # spike output segment (one DMA per TSEG steps)
# Output segment schedule: uniform 16-step segments, except the final 16
# steps go out as two 8-step segments so only ~8 steps of extraction+DMA
# sit on the kernel tail.  Device out is a flat [128, 64*T] u8 buffer,
# written contiguously segment by segment in (b, c, tl) order.
SEGS = [(s, s + TSEG) for s in range(0, T - TSEG, TSEG)] + [
    (T - TSEG, T - 4),
    (T - 4, T),
]
SEG_END = {hi: lo for lo, hi in SEGS}
F32 = mybir.dt.float32
U8 = mybir.dt.uint8

CURRENT_DECAY = 0.25
VOLTAGE_DECAY = 0.03

# Column split of the vol loop's 64 state columns (flattened (chunk,
# batch)) into two independent serial chains on DVE: A [0, CA) and
# B [CA, 64), interleaved 1A 1B 2A 2B so B's exec hides A's SBUF
# write-ack latency.  (GpSimd cannot run scalar_tensor_tensor — the
# neuronxcc backend rejects TensorScalarPtr on Pool — so the serial
# chains are DVE-only; Pool still runs the delta subtracts.)
CA = 32

# fp32r streams fp32 through the PE at bf16 rate (4x faster than plain fp32
# matmul); numerics differ slightly from fp32 — gated on a HW accuracy check.
MM_F32R = False


def _body(tc, x, v, g, out):
    nc = tc.nc
    Alu = mybir.AluOpType
    Act = mybir.ActivationFunctionType

    import contextlib

    with contextlib.ExitStack() as ctx:
        consts = ctx.enter_context(tc.tile_pool(name="consts", bufs=1))
        big = ctx.enter_context(tc.tile_pool(name="big", bufs=1))
        wp = ctx.enter_context(tc.tile_pool(name="wp", bufs=1))
        psT = ctx.enter_context(tc.tile_pool(name="psT", bufs=2, space="PSUM"))
        psZ = ctx.enter_context(tc.tile_pool(name="psZ", bufs=6, space="PSUM"))
        pvolS = ctx.enter_context(tc.tile_pool(name="pvolS", bufs=8))
        pstage = ctx.enter_context(tc.tile_pool(name="pstage", bufs=4))

        # ---- constants + single ACT table: Square/Rsqrt/Copy/Sign/Relu all
        # live in the 'reciprocal_sqrt_and_small' ACT function set, so the
        # two warm-up ops below trigger exactly ONE LoadActFuncSet. ----
        decD = consts.tile([I, 1], F32)
        nc.vector.memset(decD[:], 1.0 - CURRENT_DECAY)
        neg1 = consts.tile([128, 1], F32)
        nc.vector.memset(neg1[:], -1.0)
        actwarm = consts.tile([128, 1], F32)
        nc.scalar.activation(actwarm[:], neg1[:], Act.Square)
        nc.scalar.activation(actwarm[:], actwarm[:], Act.Sqrt)

        # ---- DMA queue order (transfers serialize on the one HW queue, so
        # order = priority): x[t<64] unblocks the first-half delta prep, the
        # two v halves unblock transposes + squares, then the rest of x in
        # two chunks so the GpSimd second-half prep can start early. ----
        xs = big.tile([I, BL * T], F32)
        x3 = xs[:].rearrange("p (b t) -> p b t", b=BL)
        delta = big.tile([I, BL * T], F32)
        d3 = delta[:].rearrange("p (b t) -> p b t", b=BL)
        TH = TBLK // 2  # DVE preps t < TH; GpSimd subs the rest
        T2 = 224  # split point of the GpSimd subs / x tail DMA
        # all inputs on the SP DMA queue, in criticality order (the model
        # serializes all DMA transfers): x[t<64] first (unblocks delta
        # prep), then the v halves (squares/transposes), then the x tail
        xr = x.rearrange("b i t -> i b t")
        vt = wp.tile([128, NCH * I], F32)
        vt3 = vt[:].rearrange("p (c i) -> p c i", c=NCH)
        vr = v.rearrange("(c p) i -> p c i", p=128)
        gt = wp.tile([128, NCH], F32)
        nc.sync.dma_start(x3[:, :, 0:TH], xr[:, :, 0:TH])
        for cq in range(0, NCH, 2):
            nc.sync.dma_start(vt3[:, cq : cq + 2, :], vr[:, cq : cq + 2, :])
        nc.sync.dma_start(gt[:], g.rearrange("(c p) -> p c", p=128))
        nc.sync.dma_start(x3[:, :, TH:T2], xr[:, :, TH:T2])
        nc.sync.dma_start(x3[:, :, T2:T], xr[:, :, T2:T])

        # ---- first-block delta + cur-delta scan (cur-delta: scan the
        # 0.75 recurrence on delta before the matmul; W.(scan delta) ==
        # scan (W.delta) by linearity).  Both on DVE, interleaved per
        # batch: the scan follows its subtract in-order with no
        # cross-engine semaphore, so the first-block prep finishes ~1.2us
        # sooner than with the subtracts on GpSimd. ----
        for b in range(BL):
            nc.vector.tensor_copy(d3[:, b, 0:1], x3[:, b, 0:1])
            nc.vector.tensor_tensor(
                out=d3[:, b, 1:TH], in0=x3[:, b, 1:TH], in1=x3[:, b, 0 : TH - 1],
                op=Alu.subtract,
            )
            seg = delta[:, b * T : b * T + TH]
            nc.vector.tensor_tensor_scan(
                seg, decD[:].to_broadcast([I, TH]), seg, 0.0, Alu.mult, Alu.add
            )
        # ---- weight-norm chain: squares on DVE (right after the delta
        # first-half, in parallel with the ACT wT copies), rsqrt in one ACT
        # op, final scale mult back on DVE. ----
        ss = wp.tile([128, NCH], F32)  # row sum-of-squares
        sq = wp.tile([128, I], F32)  # scratch
        for c in range(NCH):
            nc.vector.scalar_tensor_tensor(
                sq[:], vt3[:, c, :], 1.0, vt3[:, c, :], Alu.mult, Alu.mult,
                accum_out=ss[:, c : c + 1],
            )
        ident = consts.tile([128, 128], F32)
        make_identity(nc, ident[:])  # pool op, emitted before the pool prep

        # PE HAM warm-up: dummy matmuls during the input DMA so the real
        # matmuls run at 2.4GHz from the start (HAM un-throttles after
        # ~3.4us of sustained PE activity; 8 x 427ns covers it). PE is
        # in-order, so more warm-ups would delay the wT transposes.
        for _ in range(8):
            wps = psZ.tile([128, BL * TBLK], F32, tag="ps")
            nc.tensor.matmul(
                wps[:, 0:128], lhsT=ident[:], rhs=ident[:],
                start=True, stop=True,
            )

        # wT transposes + copies, with the norm-chain tail WEDGED between
        # the first and second half of the wc copies: reciprocal on DVE
        # (Act.Rsqrt is rejected by bass for HW accuracy), sqrt on ACT
        # right after wc c0-3 (ACT is in-order; emitting sqrt after all 8
        # wc copies would delay `scale` and the first z copies), scale
        # mult on DVE.
        inv = wp.tile([128, NCH], F32)
        rs = wp.tile([128, NCH], F32)
        scale = wp.tile([128, NCH], F32)
        wT = []  # per-chunk [126, 128] tiles of v^T
        for c in range(NCH):
            pt = psT.tile([I, 128], F32)
            nc.tensor.transpose(pt[:], vt3[:, c, :], ident[:])
            wc = wp.tile([I, 128], F32, tag=f"wT{c}")
            nc.scalar.copy(wc[:], pt[:])
            wT.append(wc)
        nc.vector.reciprocal(inv[:], ss[:])
        nc.scalar.sqrt(rs[:], inv[:])  # rsqrt(sum v^2)
        nc.vector.tensor_tensor(
            out=scale[:], in0=rs[:], in1=gt[:], op=Alu.mult
        )

        # ---- rest of the delta SUBTRACTS on GpSimd, in two t-ranges keyed
        # to the two x tail DMAs.  The matching scans are DVE-only; they
        # are emitted lazily inside the vol loop (see _pending_scans) so
        # they don't delay the loop start — block-1 matmuls don't need
        # them until ~25us into the loop.
        for rlo, rhi in ((TH, T2), (T2, T)):
            for b in range(BL):
                nc.gpsimd.tensor_tensor(
                    out=d3[:, b, rlo:rhi],
                    in0=x3[:, b, rlo:rhi],
                    in1=x3[:, b, rlo - 1 : rhi - 1],
                    op=Alu.subtract,
                )

        def _emit_tail_scan(b, rlo, rhi):
            seg = delta[:, b * T + rlo : b * T + rhi]
            carry = delta[:, b * T + rlo - 1 : b * T + rlo]
            nc.vector.tensor_tensor_scan(
                seg, decD[:].to_broadcast([I, rhi - rlo]), seg, carry,
                Alu.mult, Alu.add,
            )

        # ---- cur = (v^T . cur-delta), scaled by g/||v|| on the PSUM->SBUF
        # copy. One z tile per t-block of TBLK steps, layout [p, (c b tl)].
        # Matmul windows enumerate (tl, b) columns via a strided rhs AP on
        # delta.  Block 0 is emitted before the vol loop (in quarters, the
        # first quarter's copies on DVE); blocks 1+ are emitted a few loop
        # steps in, AFTER the lazily-emitted tail scans that produce their
        # rhs data. ----
        dly = delta[:].rearrange("p (b t) -> p t b", b=BL)  # [126, T, BL]
        ztiles = [
            big.tile([128, NCH * BL * TBLK], F32, tag=f"z{tb}", name=f"zt{tb}")
            for tb in range(NTB)
        ]

        def emit_zblock(tb, windows):
            zv = ztiles[tb][:].rearrange(
                "p (c b tl) -> p c tl b", c=NCH, b=BL
            )
            for wlo, whi in windows:
                ww = whi - wlo
                for c in range(NCH):
                    ps = psZ.tile([128, BL * TBLK], F32, tag="ps")
                    mm_lhs = wT[c][:]
                    mm_rhs = dly[:, tb * TBLK + wlo : tb * TBLK + whi, :]
                    if MM_F32R:
                        mm_lhs = mm_lhs.bitcast(mybir.dt.float32r)
                        mm_rhs = mm_rhs.bitcast(mybir.dt.float32r)
                    nc.tensor.matmul(
                        ps[:, : ww * BL], lhsT=mm_lhs, rhs=mm_rhs,
                        start=True, stop=True,
                    )
                    # psum cols are (tl, b); write to z at (b*TBLK + tl).
                    # The very first quarter's copies run on DVE (in its
                    # queue right before the vol loop, in parallel with the
                    # ACT wc copies) so the loop isn't gated on the ACT
                    # queue draining; everything else copies on ACT.
                    ps_v = ps[:, : ww * BL].rearrange("p (tl b) -> p tl b", b=BL)
                    if tb == 0 and wlo == 0 and c < NCH // 2:
                        nc.vector.tensor_scalar_mul(
                            zv[:, c, wlo:whi, :], ps_v, scale[:, c : c + 1]
                        )
                    else:
                        nc.scalar.activation(
                            zv[:, c, wlo:whi, :], ps_v,
                            Act.Copy, scale=scale[:, c : c + 1],
                        )

        # only the first half of block 0 (t < TH = 32) has scanned delta
        # before the loop starts; the second half's windows are emitted a
        # few steps into the loop, after the first injected scan.
        emit_zblock(0, ((0, 16), (16, 32)))

        # ---- vol loop: vol_pre overwrites the cur column of z in place.
        # Two independent serial chains over the 64 flattened (chunk,
        # batch) state columns, both on DVE (A [0, CA), B [CA, 64)),
        # interleaved 1A 1B 2A 2B: B's exec hides A's write-ack latency.
        groups = [
            ("A", 0, CA, nc.vector),
            ("B", CA, 64, nc.vector),
        ]
        volS = {g: None for g, _, _, _ in groups}
        vdec = 1.0 - VOLTAGE_DECAY

        # out DRAM layout is [p, ts, b, c, tl16] (host re-transposes): a
        # 16-step flush then lands as ONE DMA whose (b c tl) span is
        # contiguous per partition -> 128 descriptors instead of 8192.
        out5 = out

        def emit_step(t):
            tb, tl = divmod(t, TBLK)
            zcb = ztiles[tb][:].rearrange("p (cb tl) -> p cb tl", cb=64)
            # vol_pre = vdec * vol + cur_t   (written over cur_t).
            # t=0: vol_pre = cur_0 is already in place — skip the op.
            if t > 0:
                for g, lo, hi, eng in groups:
                    eng.scalar_tensor_tensor(
                        zcb[:, lo:hi, tl],
                        volS[g][:],
                        vdec,
                        zcb[:, lo:hi, tl],
                        Alu.mult,
                        Alu.add,
                    )
            # vol = (vol_pre < 1) * vol_pre   (hard reset); the state after
            # the last step is never consumed — skip it.
            if t < T - 1:
                for g, lo, hi, eng in groups:
                    vt = pvolS.tile([128, hi - lo], F32, tag=f"volS{g}")
                    volS[g] = vt
                    eng.scalar_tensor_tensor(
                        vt[:],
                        zcb[:, lo:hi, tl],
                        1.0,
                        zcb[:, lo:hi, tl],
                        Alu.is_lt,
                        Alu.mult,
                    )
            # spikes on ACT (off the DVE path): at each segment boundary,
            # Sign -> Relu -> one 128-descriptor DMA of the finished
            # segment (contiguous (b c tl) span in the flat out buffer).
            if (t + 1) in SEG_END:
                slo = SEG_END[t + 1]
                w = t + 1 - slo
                lo = slo - tb * TBLK
                hi = lo + w
                zcf = ztiles[tb][:].rearrange(
                    "p (c b tl) -> p c b tl", c=NCH, b=BL
                )
                ostage = pstage.tile([128, 64 * w], U8, tag=f"os{w}")
                o3 = ostage[:].rearrange(
                    "p (b c tl) -> p b c tl", b=BL, c=NCH
                )
                # spikes are exact 0/1: Sign then Relu (which also narrows
                # to uint8 -> 4x fewer DMA bytes; host widens). A DVE
                # is_ge->uint8 shortcut matched in CoreSim but was WRONG
                # on hardware — keep ACT.
                sstage = pstage.tile([128, 64 * w], F32, tag=f"ss{w}")
                s3 = sstage[:].rearrange(
                    "p (b c tl) -> p c b tl", b=BL, c=NCH
                )
                nc.scalar.activation(
                    s3, zcf[:, :, :, lo:hi], Act.Sign, bias=neg1[:]
                )
                nc.scalar.activation(ostage[:], sstage[:], Act.Relu)
                nc.sync.dma_start(out5[:, 64 * slo : 64 * (t + 1)], o3)

        # The rest of the work is interleaved into the loop emission, in
        # dependency order: the 16 tail scans are injected one per two
        # steps ([TH,T2) as soon as its x DMA + subs can land, [T2,T)
        # later); the second half of z block 0, then block 1, then blocks
        # 2-5 are emitted once the scans covering their rhs are in.
        SPLIT_0B = 11
        SPLIT_B1 = 13
        SPLIT_REST = 22
        for t in range(T):
            emit_step(t)
            if 2 <= t < 2 + BL:
                _emit_tail_scan(t - 2, TH, T2)
            if 14 <= t < 14 + BL:
                _emit_tail_scan(t - 14, T2, T)
            if t == SPLIT_0B:
                emit_zblock(0, ((32, 48), (48, 64)))
            if t == SPLIT_B1:
                emit_zblock(1, ((0, TBLK),))
            if t == SPLIT_REST:
                for tb in range(2, NTB):
                    emit_zblock(tb, ((0, TBLK),))


_CACHE = {}


def _build():
    if "nc" in _CACHE:
        return _CACHE["nc"]
    nc = bacc.Bacc(
        "TRN2", target_bir_lowering=False, debug=False, num_devices=N_CORES
    )
    x = nc.dram_tensor("x", [BL, I, T], F32, kind="ExternalInput").ap()
    v = nc.dram_tensor("v", [O, I], F32, kind="ExternalInput").ap()
    g = nc.dram_tensor("g", [O], F32, kind="ExternalInput").ap()
    out = nc.dram_tensor(
        "out", [128, 64 * T], U8, kind="ExternalOutput"
    ).ap()
    with tile.TileContext(nc) as tc:
        _body(tc, x, v, g, out)
    nc.compile()
    _CACHE["nc"] = nc
    return nc


def make_in_maps(x, v_weight, g):
    xr = np.ascontiguousarray(x.reshape(B, I, T))
    v_weight = np.ascontiguousarray(v_weight)
    g = np.ascontiguousarray(g)
    return [
        {
            "x": np.ascontiguousarray(xr[c * BL : (c + 1) * BL]),
            "v": v_weight,
            "g": g,
        }
        for c in range(N_CORES)
    ]


def kernel(x, v_weight, g):
    nc = _build()
    in_maps = make_in_maps(
        np.asarray(x, dtype=np.float32),
        np.asarray(v_weight, dtype=np.float32),
        np.asarray(g, dtype=np.float32),
    )
    last_err = None
    for _attempt in range(3):  # retry: a prior tenant can leave a core wedged
        try:
            res = run_bass_kernel_spmd(nc, in_maps, list(range(N_CORES))).results
            # device out is a flat [128, 64*T] buffer of (b, c, tl)
            # segments per SEGS; host untransposes to [b, o=c*128+p, t]
            parts = []
            for core in range(N_CORES):
                arr = res[core]["out"]  # [128, 64*T] u8
                full = np.empty((BL, O, T), np.uint8)
                o_view = full.reshape(BL, NCH, 128, T)
                for lo, hi in SEGS:
                    w = hi - lo
                    seg = arr[:, 64 * lo : 64 * hi].reshape(128, BL, NCH, w)
                    # seg[p, b, c, tl] -> o_view[b, c, p, lo:hi]
                    o_view[:, :, :, lo:hi] = np.transpose(seg, (1, 2, 0, 3))
                parts.append(full)
            return np.concatenate(parts, axis=0).astype(np.float32)
        except Exception as e:  # noqa: BLE001
            last_err = e
    raise last_err

